# revision 30
# baseline (speedup 1.0000x reference)
"""Causal single-head attention forward on 8 Trainium2 NeuronCores.

Problem: x:(512,256,384) f32, Wq/Wk/Wv:(384,64) f32
  k = x@Wk; q = x@Wq; v = x@Wv
  wei = softmax(mask(q k^T / sqrt(384)))
  out = wei @ v                       -> (512, 256, 64) f32

Strategy: pure data parallel over batch (64 batches/core, no collectives).
Per core, batches are processed in macro-iterations of 2 batches, software
pipelined three deep: iteration i issues load(i+2) / transpose(i+1) /
compute(i) so every engine's FIFO has next-macro prep ahead of this
macro's tail ops.

Per macro:
  - load x naturally [T-part, C-free] (one contiguous cast-DMA f32->bf16)
  - transpose on PE via REAL matmuls (x block stationary, identity
    streaming) -- regular matmuls engage FWL and keep the HAM clock warm,
    unlike transpose-mode (~275ns/blk and no warmth credit)
  - packed [Wq|Wk... actually Wq solo and [Wk|Wv]] projections as in:
    qT [64,512] and kvT [128,512] over 3 C-chunks
  - S computed TRANSPOSED: sT[k, q] = kT.T @ qT so exp(sT) = pT is
    directly the lhsT of the PV matmul (no P transpose needed)
  - causal: block (k1, q0) never computed; diagonal blocks masked by a
    0/1 upper-triangular multiply after exp (one DVE op per batch via a
    strided 2-block view)
  - softmax denominators via a ones-column appended to v (col 64 of the
    PV matmul accumulates row sums); both batches' PV land in ONE psum
    tile so a single reciprocal + single broadcast-multiply normalize
    the whole macro.
All matmuls bf16 inputs with fp32 PSUM accumulation.
"""

import os
from contextlib import ExitStack

import numpy as np

import concourse.bass as bass
import concourse.mybir as mybir
import concourse.tile as tile
from concourse import bacc
from concourse.bass_utils import run_bass_kernel_spmd
from concourse.masks import make_identity

B, T, C, H = 512, 256, 384, 64
N_CORES = 8
B_LOC = B // N_CORES          # 64 batches per core
MACRO = 2                     # batches per macro-iteration
N_MACRO = B_LOC // MACRO      # 32
NC_CHUNKS = C // 128          # 3
SCALE = float(C) ** -0.5

F32 = mybir.dt.float32
F32R = mybir.dt.float32r
BF16 = mybir.dt.bfloat16


def build_attention_kernel(ctx, tc, out_d, x_d, wq_d, wk_d, wv_d, reps=1,
                           variant="full"):
    nc = tc.nc
    pool = lambda *a, **k: ctx.enter_context(tc.tile_pool(*a, **k))

    # ---------------- constants / weights (once) ----------------
    cpool = pool(name="const", bufs=1)
    ident = cpool.tile([128, 128], BF16, tag="ident")
    make_identity(nc, ident[:, :])

    # upper-triangular (incl diag) 0/1 mask in bf16: mask[j, i] = 1 if i >= j
    maskT = cpool.tile([128, 128], BF16, tag="maskT")
    nc.gpsimd.memset(maskT[:, :], 1.0)
    nc.gpsimd.affine_select(
        out=maskT[:, :],
        in_=maskT[:, :],
        compare_op=mybir.AluOpType.is_ge,
        fill=0.0,
        base=0,
        pattern=[[1, 128]],       # iota = -j + i  (channel j, free i)
        channel_multiplier=-1,
    )

    # weights: DRAM (384,64) -> staging f32 [128, 3, 64] -> packed bf16
    wst = cpool.tile([128, 3 * NC_CHUNKS, 64], F32, tag="wstage")
    for i, wd in enumerate((wq_d, wk_d, wv_d)):
        nc.sync.dma_start(
            wst[:, i * NC_CHUNKS:(i + 1) * NC_CHUNKS, :],
            wd.rearrange("(n p) h -> p n h", p=128),
        )
    # bf16 weights [128(C within chunk), chunk, *]; Wq pre-scaled by C^-0.5.
    # wkv packs [Wk | Wv] along the output dim -> kv projection in one chain.
    wq = cpool.tile([128, NC_CHUNKS, 64], BF16, tag="wq")
    nc.vector.tensor_scalar_mul(wq[:, :, :], wst[:, 0:NC_CHUNKS, :], SCALE)
    wkv = cpool.tile([128, NC_CHUNKS, 128], BF16, tag="wkv")
    nc.vector.tensor_copy(wkv[:, :, 0:64], wst[:, NC_CHUNKS:2 * NC_CHUNKS, :])
    nc.vector.tensor_copy(wkv[:, :, 64:128], wst[:, 2 * NC_CHUNKS:3 * NC_CHUNKS, :])

    # ---------------- pools ----------------
    xf_pool = pool(name="xf", bufs=4)       # bf16 natural x
    xts_pool = pool(name="xts", bufs=9)     # bf16 xT in SBUF (3/macro)
    qs_pool = pool(name="qs", bufs=3)       # bf16 qT in SBUF [64, 512]
    kvs_pool = pool(name="kvs", bufs=3)     # bf16 [kT|vT] in SBUF [128, 512]
    vs_pool = pool(name="vs", bufs=3)       # bf16 v (+ones col)
    pt_pool = pool(name="pt", bufs=4)       # bf16 pT
    os_pool = pool(name="os", bufs=3)       # f32 out staging
    rec_pool = pool(name="rec", bufs=4)     # f32 reciprocal

    RD1_OPS = os.environ.get("KERNEL_RD1_OPS", "") == "1"

    xtp_pool = pool(name="xtp", bufs=2, space="PSUM")   # xT psum
    qp_pool = pool(name="qp", bufs=1, space="PSUM")     # qT psum [64, 512]
    kvp_pool = pool(name="kvp", bufs=1, space="PSUM")   # kvT psum [128, 512]
    stp_pool = pool(name="stp", bufs=2, space="PSUM")   # sT psum
    if RD1_OPS:
        vop_pool = pool(name="vtp", bufs=1, space="PSUM")
        op_pool = pool(name="op", bufs=1, space="PSUM")
    else:
        # vtp and op share one double-buffered ring (lifetimes interleave)
        vop_pool = pool(name="vop", bufs=2, space="PSUM")
        op_pool = vop_pool

    x_rm = x_d.rearrange("(m b) (n p) c -> m p b n c", p=128, b=MACRO)
    out_rm = out_d.rearrange("(m b) (n p) h -> m p b n h", p=128, b=MACRO)

    SPLIT_DMA = os.environ.get("KERNEL_SPLIT_DMA", "") == "1"
    state = {}

    def stage_load(j):
        xf = xf_pool.tile([128, MACRO, 2, C], BF16, tag="xf")
        if SPLIT_DMA:
            for b in range(MACRO):
                nc.gpsimd.dma_start(xf[:, b, :, :], x_rm[j, :, b, :, :])
        else:
            nc.gpsimd.dma_start(xf[:, :, :, :], x_rm[j, :, :, :, :])
        state[j] = {"xf": xf}

    def stage_transpose(j):
        # x natural -> xT on PE as true matmuls (x stationary, I streaming)
        xf = state[j]["xf"]
        xts = []
        for c in range(NC_CHUNKS):
            xtp = xtp_pool.tile([128, 512], F32, tag="xtp")
            for b in range(MACRO):
                for t in range(2):
                    nc.tensor.matmul(
                        xtp[:, (b * 2 + t) * 128:(b * 2 + t) * 128 + 128],
                        xf[:, b, t, c * 128:(c + 1) * 128],
                        ident[:, :],
                        start=True, stop=True,
                    )
            xt = xts_pool.tile([128, 512], BF16, tag="xts")
            nc.scalar.copy(xt[:, :], xtp[:, :])
            xts.append(xt)
        state[j]["xts"] = xts

    def stage_compute(it):
        xts = state[it]["xts"]

        # ---- projections: qT [64, 512]; [kT|vT] packed [128, 512] ----
        qp = qp_pool.tile([64, 512], F32, tag="qp")
        kvp = kvp_pool.tile([128, 512], F32, tag="kvp")
        for c in range(NC_CHUNKS):
            nc.tensor.matmul(
                qp[:, :], wq[:, c, :], xts[c][:, :],
                start=(c == 0), stop=(c == NC_CHUNKS - 1),
            )
        for c in range(NC_CHUNKS):
            nc.tensor.matmul(
                kvp[:, :], wkv[:, c, :], xts[c][:, :],
                start=(c == 0), stop=(c == NC_CHUNKS - 1),
            )
        qs = qs_pool.tile([64, 512], BF16, tag="qs")
        nc.vector.tensor_copy(qs[:, :], qp[:, :])
        kvs = kvs_pool.tile([128, 512], BF16, tag="kvs")
        nc.vector.tensor_copy(kvs[:, :], kvp[:, :])

        # ---- v natural [128(T), 64] via matmul-transpose of the vT half ----
        vtp = vop_pool.tile([128, 256], F32, tag="vop")
        for q in range(4):  # q = b*2 + t
            nc.tensor.matmul(
                vtp[:, q * 64:(q + 1) * 64],
                kvs[64:128, q * 128:(q + 1) * 128],
                ident[64:128, 64:128],
                start=True, stop=True,
            )
        vs = vs_pool.tile([128, 4, 65], BF16, tag="vs")
        nc.vector.tensor_copy(
            vs[:, :, 0:64], vtp.rearrange("p (q h) -> p q h", h=64)
        )
        nc.vector.memset(vs[:, :, 64], 1.0)

        # ---- sT for both batches, then per-batch exp/mask/PV ----
        stps = []
        for b in range(MACRO):
            qcol = b * 256
            # sT: [128, 0:256] = sT(k0, q0|q1); [128, 256:384] = sT(k1, q1)
            stp = stp_pool.tile([128, 384], F32, tag="stp")
            nc.tensor.matmul(
                stp[:, 0:256],
                kvs[0:64, qcol:qcol + 128],        # kT chunk k0 [64, 128]
                qs[:, qcol:qcol + 256],            # qT (both chunks) [64, 256]
                start=True, stop=True,
            )
            nc.tensor.matmul(
                stp[:, 256:384],
                kvs[0:64, qcol + 128:qcol + 256],  # kT chunk k1
                qs[:, qcol + 128:qcol + 256],      # qT chunk q1
                start=True, stop=True,
            )
            stps.append(stp)

        osb = os_pool.tile([128, MACRO, 2, 64], F32, tag="os")
        if RD1_OPS:
            for b in range(MACRO):
                stp = stps[b]
                pt = pt_pool.tile([128, 384], BF16, tag="pt")
                nc.scalar.activation(
                    pt[:, :], stp[:, :], mybir.ActivationFunctionType.Exp
                )
                nc.gpsimd.tensor_mul(pt[:, 0:128], pt[:, 0:128], maskT[:, :])
                nc.gpsimd.tensor_mul(
                    pt[:, 256:384], pt[:, 256:384], maskT[:, :]
                )
                op = op_pool.tile([128, 130], F32, tag="op")
                nc.tensor.matmul(
                    op[:, 0:65], pt[:, 0:128], vs[:, b * 2 + 0, :],
                    start=True, stop=True,
                )
                nc.tensor.matmul(
                    op[:, 65:130], pt[:, 128:256], vs[:, b * 2 + 0, :],
                    start=True, stop=False,
                )
                nc.tensor.matmul(
                    op[:, 65:130], pt[:, 256:384], vs[:, b * 2 + 1, :],
                    start=False, stop=True,
                )
                rec = rec_pool.tile([128, 2], F32, tag="rec")
                nc.vector.reciprocal(
                    rec[:, :],
                    op.rearrange("p (n q) -> p n q", q=65)[:, :, 64],
                )
                nc.vector.tensor_scalar_mul(
                    osb[:, b, 0, :], op[:, 0:64], rec[:, 0:1]
                )
                nc.vector.tensor_scalar_mul(
                    osb[:, b, 1, :], op[:, 65:129], rec[:, 1:2]
                )
        else:
            # both batches' PV outputs land in one psum tile [128, 2, 130]
            op = op_pool.tile([128, 2, 130], F32, tag="vop")
            for b in range(MACRO):
                stp = stps[b]
                # pT = exp(sT)  (no max subtraction: logits are O(+-3))
                pt = pt_pool.tile([128, 384], BF16, tag="pt")
                nc.scalar.activation(
                    pt[:, :], stp[:, :], mybir.ActivationFunctionType.Exp
                )
                # causal 0/1 masking of the two diagonal blocks. Pool is the
                # least-busy engine and the skewed loop issues its DMA-gen
                # work early, so masks don't queue behind it.
                if os.environ.get("KERNEL_DVE_MASK", "") == "1":
                    ptv = pt.rearrange("p (n q) -> p n q", q=128)[:, 0:3:2, :]
                    mask_b, ptv_b = bass.broadcast_tensor_aps(
                        maskT.rearrange("p (o q) -> p o q", o=1), ptv
                    )
                    nc.vector.tensor_tensor(
                        ptv, ptv_b, mask_b, mybir.AluOpType.mult
                    )
                else:
                    nc.gpsimd.tensor_mul(
                        pt[:, 0:128], pt[:, 0:128], maskT[:, :]
                    )
                    nc.gpsimd.tensor_mul(
                        pt[:, 256:384], pt[:, 256:384], maskT[:, :]
                    )

                # out = pT.T @ [v|1]: cols 0:65 = q0 (k0 only), 65:130 = q1
                nc.tensor.matmul(
                    op[:, b, 0:65], pt[:, 0:128], vs[:, b * 2 + 0, :],
                    start=True, stop=True,
                )
                nc.tensor.matmul(
                    op[:, b, 65:130], pt[:, 128:256], vs[:, b * 2 + 0, :],
                    start=True, stop=False,
                )
                nc.tensor.matmul(
                    op[:, b, 65:130], pt[:, 256:384], vs[:, b * 2 + 1, :],
                    start=False, stop=True,
                )

            # single reciprocal + single broadcast-normalize for the macro
            op_v = op.rearrange("p b (n q) -> p (b n) q", q=65)  # [128,4,65]
            rec = rec_pool.tile([128, 4], F32, tag="rec")
            nc.vector.reciprocal(rec[:, :], op_v[:, :, 64])
            rec_b, opd_b = bass.broadcast_tensor_aps(
                rec.rearrange("p (n o) -> p n o", o=1), op_v[:, :, 0:64]
            )
            nc.vector.tensor_tensor(
                osb.rearrange("p b n h -> p (b n) h"), opd_b, rec_b,
                mybir.AluOpType.mult,
            )

        # all stores on SP's HWDGE ring: nc.scalar shares the ACT engine's
        # queue and ACT is the busiest engine
        nc.sync.dma_start(out_rm[it, :, :, :, :], osb[:, :, :, :])
        del state[it]

    if reps > 1:
        rep_ctx = tc.For_i(0, reps, 1)
        rep_ctx.__enter__()

    PIPELINE = os.environ.get("KERNEL_FLAT", "") != "1"
    if PIPELINE:
        stage_load(0)
        stage_load(1)
        stage_transpose(0)
        for i in range(N_MACRO):
            if i + 2 < N_MACRO:
                stage_load(i + 2)
            if i + 1 < N_MACRO:
                stage_transpose(i + 1)
            stage_compute(i)
    else:
        for i in range(N_MACRO):
            stage_load(i)
            stage_transpose(i)
            stage_compute(i)

    if reps > 1:
        rep_ctx.__exit__(None, None, None)


_CACHED = {}


def _build(reps=1, variant="full"):
    key = (reps, variant)
    if key in _CACHED:
        return _CACHED[key]
    nc = bacc.Bacc(
        "TRN2",
        target_bir_lowering=False,
        debug=False,
        num_devices=N_CORES,
        dynamic_dma_scratch_size=int(
            os.environ.get("KERNEL_SCRATCH", "49152")
        ),
    )
    x_d = nc.dram_tensor("x", [B_LOC, T, C], F32, kind="ExternalInput").ap()
    wq_d = nc.dram_tensor("Wq", [C, H], F32, kind="ExternalInput").ap()
    wk_d = nc.dram_tensor("Wk", [C, H], F32, kind="ExternalInput").ap()
    wv_d = nc.dram_tensor("Wv", [C, H], F32, kind="ExternalInput").ap()
    out_d = nc.dram_tensor("out", [B_LOC, T, H], F32, kind="ExternalOutput").ap()
    with tile.TileContext(nc) as tc, ExitStack() as ctx:
        build_attention_kernel(
            ctx, tc, out_d, x_d, wq_d, wk_d, wv_d, reps=reps, variant=variant
        )
    nc.compile()
    _CACHED[key] = nc
    return nc


_RUNNER = {}


def _get_runner(reps=1, variant="full"):
    """Persistent jitted SPMD executor (compiles/loads the NEFF once)."""
    key = (reps, variant)
    if key in _RUNNER:
        return _RUNNER[key]

    import jax
    from jax.sharding import Mesh, PartitionSpec
    from jax.experimental.shard_map import shard_map
    from concourse import bass2jax

    nc = _build(reps, variant)
    bass2jax.install_neuronx_cc_hook()

    partition_name = (
        nc.partition_id_tensor.name if nc.partition_id_tensor else None
    )
    in_names, out_names, out_avals = [], [], []
    for alloc in nc.m.functions[0].allocations:
        if not isinstance(alloc, mybir.MemoryLocationSet):
            continue
        name = alloc.memorylocations[0].name
        if alloc.kind == "ExternalInput":
            if name != partition_name:
                in_names.append(name)
        elif alloc.kind == "ExternalOutput":
            out_names.append(name)
            out_avals.append(
                jax.core.ShapedArray(
                    tuple(alloc.tensor_shape), mybir.dt.np(alloc.dtype)
                )
            )
    n_params = len(in_names)
    all_in_names = in_names + out_names
    if partition_name is not None:
        all_in_names = all_in_names + [partition_name]

    def _body(*args):
        operands = list(args)
        if partition_name is not None:
            operands.append(bass2jax.partition_id_tensor())
        outs = bass2jax._bass_exec_p.bind(
            *operands,
            out_avals=tuple(out_avals),
            in_names=tuple(all_in_names),
            out_names=tuple(out_names),
            lowering_input_output_aliases=(),
            sim_require_finite=True,
            sim_require_nnan=True,
            nc=nc,
        )
        return tuple(outs)

    devices = jax.devices()[:N_CORES]
    mesh = Mesh(np.asarray(devices), ("core",))
    fn = jax.jit(
        shard_map(
            _body,
            mesh=mesh,
            in_specs=(PartitionSpec("core"),) * (n_params + len(out_names)),
            out_specs=(PartitionSpec("core"),) * len(out_names),
            check_rep=False,
        ),
        keep_unused=True,
    )
    zero_outs = [
        np.zeros((N_CORES * a.shape[0], *a.shape[1:]), a.dtype) for a in out_avals
    ]
    _RUNNER[key] = (fn, in_names, out_names, out_avals, zero_outs)
    return _RUNNER[key]


def _global_inputs(x, Wk, Wq, Wv):
    """Concatenated per-core inputs keyed by BIR input name."""
    reps = {
        "x": np.ascontiguousarray(x, dtype=np.float32),
        "Wq": np.tile(np.asarray(Wq, np.float32), (N_CORES, 1)),
        "Wk": np.tile(np.asarray(Wk, np.float32), (N_CORES, 1)),
        "Wv": np.tile(np.asarray(Wv, np.float32), (N_CORES, 1)),
    }
    return reps


def kernel(x, Wk, Wq, Wv):
    x = np.asarray(x, dtype=np.float32)
    fn, in_names, out_names, out_avals, zero_outs = _get_runner()
    gi = _global_inputs(x, Wk, Wq, Wv)
    args = [gi[n] for n in in_names] + zero_outs
    outs = fn(*args)
    out = np.asarray(outs[out_names.index("out")])
    return out.astype(np.float32)


# revision 32
# speedup vs baseline: 1.0485x; 1.0485x over previous
"""Causal single-head attention forward on 8 Trainium2 NeuronCores.

Problem: x:(512,256,384) f32, Wq/Wk/Wv:(384,64) f32
  k = x@Wk; q = x@Wq; v = x@Wv
  wei = softmax(mask(q k^T / sqrt(384)))
  out = wei @ v                       -> (512, 256, 64) f32

Strategy: pure data parallel over batch (64 batches/core, no collectives).
Per core, batches are processed in macro-iterations of 2 batches, software
pipelined three deep: iteration i issues load(i+2) / transpose(i+1) /
compute(i) so every engine's FIFO has next-macro prep ahead of this
macro's tail ops.

Per macro:
  - load x naturally [T-part, C-free] (one contiguous cast-DMA f32->bf16)
  - transpose on PE via REAL matmuls (x block stationary, identity
    streaming) -- regular matmuls engage FWL and keep the HAM clock warm,
    unlike transpose-mode (~275ns/blk and no warmth credit)
  - packed [Wq|Wk... actually Wq solo and [Wk|Wv]] projections as in:
    qT [64,512] and kvT [128,512] over 3 C-chunks
  - S computed TRANSPOSED: sT[k, q] = kT.T @ qT so exp(sT) = pT is
    directly the lhsT of the PV matmul (no P transpose needed)
  - causal: block (k1, q0) never computed; diagonal blocks masked by a
    0/1 upper-triangular multiply after exp (one DVE op per batch via a
    strided 2-block view)
  - softmax denominators via a ones-column appended to v (col 64 of the
    PV matmul accumulates row sums); both batches' PV land in ONE psum
    tile so a single reciprocal + single broadcast-multiply normalize
    the whole macro.
All matmuls bf16 inputs with fp32 PSUM accumulation.
"""

import os
from contextlib import ExitStack

import numpy as np

import concourse.bass as bass
import concourse.mybir as mybir
import concourse.tile as tile
from concourse import bacc
from concourse.bass_utils import run_bass_kernel_spmd
from concourse.masks import make_identity

B, T, C, H = 512, 256, 384, 64
N_CORES = 8
B_LOC = B // N_CORES          # 64 batches per core
MACRO = 2                     # batches per macro-iteration
N_MACRO = B_LOC // MACRO      # 32
NC_CHUNKS = C // 128          # 3
SCALE = float(C) ** -0.5

F32 = mybir.dt.float32
F32R = mybir.dt.float32r
BF16 = mybir.dt.bfloat16


def build_attention_kernel(ctx, tc, out_d, x_d, wq_d, wk_d, wv_d, reps=1,
                           variant="full"):
    nc = tc.nc
    pool = lambda *a, **k: ctx.enter_context(tc.tile_pool(*a, **k))

    # ---------------- constants / weights (once) ----------------
    cpool = pool(name="const", bufs=1)
    ident = cpool.tile([128, 128], BF16, tag="ident")
    make_identity(nc, ident[:, :])

    # upper-triangular (incl diag) 0/1 mask in bf16: mask[j, i] = 1 if i >= j
    maskT = cpool.tile([128, 128], BF16, tag="maskT")
    nc.gpsimd.memset(maskT[:, :], 1.0)
    nc.gpsimd.affine_select(
        out=maskT[:, :],
        in_=maskT[:, :],
        compare_op=mybir.AluOpType.is_ge,
        fill=0.0,
        base=0,
        pattern=[[1, 128]],       # iota = -j + i  (channel j, free i)
        channel_multiplier=-1,
    )

    # weights: DRAM (384,64) -> staging f32 [128, 3, 64] -> packed bf16
    wst = cpool.tile([128, 3 * NC_CHUNKS, 64], F32, tag="wstage")
    for i, wd in enumerate((wq_d, wk_d, wv_d)):
        nc.sync.dma_start(
            wst[:, i * NC_CHUNKS:(i + 1) * NC_CHUNKS, :],
            wd.rearrange("(n p) h -> p n h", p=128),
        )
    # bf16 weights [128(C within chunk), chunk, *]; Wq pre-scaled by C^-0.5.
    # wkv packs [Wk | Wv] along the output dim -> kv projection in one chain.
    wq = cpool.tile([128, NC_CHUNKS, 64], BF16, tag="wq")
    nc.vector.tensor_scalar_mul(wq[:, :, :], wst[:, 0:NC_CHUNKS, :], SCALE)
    wkv = cpool.tile([128, NC_CHUNKS, 128], BF16, tag="wkv")
    nc.vector.tensor_copy(wkv[:, :, 0:64], wst[:, NC_CHUNKS:2 * NC_CHUNKS, :])
    nc.vector.tensor_copy(wkv[:, :, 64:128], wst[:, 2 * NC_CHUNKS:3 * NC_CHUNKS, :])

    # ---------------- pools ----------------
    xf_pool = pool(name="xf", bufs=6)       # bf16 natural x
    xts_pool = pool(name="xts", bufs=12)    # bf16 xT in SBUF (3/macro)
    qs_pool = pool(name="qs", bufs=4)       # bf16 qT in SBUF [64, 512]
    kvs_pool = pool(name="kvs", bufs=4)     # bf16 [kT|vT] in SBUF [128, 512]
    vs_pool = pool(name="vs", bufs=4)       # bf16 v (+ones col)
    pt_pool = pool(name="pt", bufs=6)       # bf16 pT
    os_pool = pool(name="os", bufs=4)       # f32 out staging
    rec_pool = pool(name="rec", bufs=4)     # f32 reciprocal

    RD1_OPS = os.environ.get("KERNEL_RD1_OPS", "") == "1"

    xtp_pool = pool(name="xtp", bufs=2, space="PSUM")   # xT psum
    qp_pool = pool(name="qp", bufs=1, space="PSUM")     # qT psum [64, 512]
    kvp_pool = pool(name="kvp", bufs=1, space="PSUM")   # kvT psum [128, 512]
    stp_pool = pool(name="stp", bufs=2, space="PSUM")   # sT psum
    if RD1_OPS:
        vop_pool = pool(name="vtp", bufs=1, space="PSUM")
        op_pool = pool(name="op", bufs=1, space="PSUM")
    else:
        # vtp and op share one double-buffered ring (lifetimes interleave)
        vop_pool = pool(name="vop", bufs=2, space="PSUM")
        op_pool = vop_pool

    x_rm = x_d.rearrange("(m b) (n p) c -> m p b n c", p=128, b=MACRO)
    out_rm = out_d.rearrange("(m b) (n p) h -> m p b n h", p=128, b=MACRO)

    SPLIT_DMA = os.environ.get("KERNEL_SPLIT_DMA", "") == "1"
    state = {}

    def stage_load(j):
        xf = xf_pool.tile([128, MACRO, 2, C], BF16, tag="xf")
        if SPLIT_DMA:
            for b in range(MACRO):
                nc.gpsimd.dma_start(xf[:, b, :, :], x_rm[j, :, b, :, :])
        else:
            nc.gpsimd.dma_start(xf[:, :, :, :], x_rm[j, :, :, :, :])
        state[j] = {"xf": xf}

    def stage_transpose(j):
        # x natural -> xT on PE as true matmuls (x stationary, I streaming)
        xf = state[j]["xf"]
        xts = []
        for c in range(NC_CHUNKS):
            xtp = xtp_pool.tile([128, 512], F32, tag="xtp")
            for b in range(MACRO):
                for t in range(2):
                    nc.tensor.matmul(
                        xtp[:, (b * 2 + t) * 128:(b * 2 + t) * 128 + 128],
                        xf[:, b, t, c * 128:(c + 1) * 128],
                        ident[:, :],
                        start=True, stop=True,
                    )
            xt = xts_pool.tile([128, 512], BF16, tag="xts")
            nc.scalar.copy(xt[:, :], xtp[:, :])
            xts.append(xt)
        state[j]["xts"] = xts

    def stage_compute(it):
        xts = state[it]["xts"]

        # ---- projections: qT [64, 512]; [kT|vT] packed [128, 512] ----
        qp = qp_pool.tile([64, 512], F32, tag="qp")
        kvp = kvp_pool.tile([128, 512], F32, tag="kvp")
        for c in range(NC_CHUNKS):
            nc.tensor.matmul(
                qp[:, :], wq[:, c, :], xts[c][:, :],
                start=(c == 0), stop=(c == NC_CHUNKS - 1),
            )
        for c in range(NC_CHUNKS):
            nc.tensor.matmul(
                kvp[:, :], wkv[:, c, :], xts[c][:, :],
                start=(c == 0), stop=(c == NC_CHUNKS - 1),
            )
        qs = qs_pool.tile([64, 512], BF16, tag="qs")
        nc.vector.tensor_copy(qs[:, :], qp[:, :])
        kvs = kvs_pool.tile([128, 512], BF16, tag="kvs")
        nc.vector.tensor_copy(kvs[:, :], kvp[:, :])

        # ---- v natural [128(T), 64] via matmul-transpose of the vT half ----
        vtp = vop_pool.tile([128, 256], F32, tag="vop")
        for q in range(4):  # q = b*2 + t
            nc.tensor.matmul(
                vtp[:, q * 64:(q + 1) * 64],
                kvs[64:128, q * 128:(q + 1) * 128],
                ident[64:128, 64:128],
                start=True, stop=True,
            )
        vs = vs_pool.tile([128, 4, 65], BF16, tag="vs")
        nc.vector.tensor_copy(
            vs[:, :, 0:64], vtp.rearrange("p (q h) -> p q h", h=64)
        )
        nc.vector.memset(vs[:, :, 64], 1.0)

        # ---- sT for both batches, then per-batch exp/mask/PV ----
        stps = []
        for b in range(MACRO):
            qcol = b * 256
            # sT: [128, 0:256] = sT(k0, q0|q1); [128, 256:384] = sT(k1, q1)
            stp = stp_pool.tile([128, 384], F32, tag="stp")
            nc.tensor.matmul(
                stp[:, 0:256],
                kvs[0:64, qcol:qcol + 128],        # kT chunk k0 [64, 128]
                qs[:, qcol:qcol + 256],            # qT (both chunks) [64, 256]
                start=True, stop=True,
            )
            nc.tensor.matmul(
                stp[:, 256:384],
                kvs[0:64, qcol + 128:qcol + 256],  # kT chunk k1
                qs[:, qcol + 128:qcol + 256],      # qT chunk q1
                start=True, stop=True,
            )
            stps.append(stp)

        osb = os_pool.tile([128, MACRO, 2, 64], F32, tag="os")
        if RD1_OPS:
            for b in range(MACRO):
                stp = stps[b]
                pt = pt_pool.tile([128, 384], BF16, tag="pt")
                nc.scalar.activation(
                    pt[:, :], stp[:, :], mybir.ActivationFunctionType.Exp
                )
                nc.gpsimd.tensor_mul(pt[:, 0:128], pt[:, 0:128], maskT[:, :])
                nc.gpsimd.tensor_mul(
                    pt[:, 256:384], pt[:, 256:384], maskT[:, :]
                )
                op = op_pool.tile([128, 130], F32, tag="op")
                nc.tensor.matmul(
                    op[:, 0:65], pt[:, 0:128], vs[:, b * 2 + 0, :],
                    start=True, stop=True,
                )
                nc.tensor.matmul(
                    op[:, 65:130], pt[:, 128:256], vs[:, b * 2 + 0, :],
                    start=True, stop=False,
                )
                nc.tensor.matmul(
                    op[:, 65:130], pt[:, 256:384], vs[:, b * 2 + 1, :],
                    start=False, stop=True,
                )
                rec = rec_pool.tile([128, 2], F32, tag="rec")
                nc.vector.reciprocal(
                    rec[:, :],
                    op.rearrange("p (n q) -> p n q", q=65)[:, :, 64],
                )
                nc.vector.tensor_scalar_mul(
                    osb[:, b, 0, :], op[:, 0:64], rec[:, 0:1]
                )
                nc.vector.tensor_scalar_mul(
                    osb[:, b, 1, :], op[:, 65:129], rec[:, 1:2]
                )
        else:
            # both batches' PV outputs land in one psum tile [128, 2, 130]
            op = op_pool.tile([128, 2, 130], F32, tag="vop")
            for b in range(MACRO):
                stp = stps[b]
                # pT = exp(sT)  (no max subtraction: logits are O(+-3))
                pt = pt_pool.tile([128, 384], BF16, tag="pt")
                nc.scalar.activation(
                    pt[:, :], stp[:, :], mybir.ActivationFunctionType.Exp
                )
                # causal 0/1 masking of the two diagonal blocks. Pool is the
                # least-busy engine and the skewed loop issues its DMA-gen
                # work early, so masks don't queue behind it.
                if os.environ.get("KERNEL_DVE_MASK", "1") == "1":
                    ptv = pt.rearrange("p (n q) -> p n q", q=128)[:, 0:3:2, :]
                    mask_b, ptv_b = bass.broadcast_tensor_aps(
                        maskT.rearrange("p (o q) -> p o q", o=1), ptv
                    )
                    nc.vector.tensor_tensor(
                        ptv, ptv_b, mask_b, mybir.AluOpType.mult
                    )
                else:
                    nc.gpsimd.tensor_mul(
                        pt[:, 0:128], pt[:, 0:128], maskT[:, :]
                    )
                    nc.gpsimd.tensor_mul(
                        pt[:, 256:384], pt[:, 256:384], maskT[:, :]
                    )

                # out = pT.T @ [v|1]: cols 0:65 = q0 (k0 only), 65:130 = q1
                nc.tensor.matmul(
                    op[:, b, 0:65], pt[:, 0:128], vs[:, b * 2 + 0, :],
                    start=True, stop=True,
                )
                nc.tensor.matmul(
                    op[:, b, 65:130], pt[:, 128:256], vs[:, b * 2 + 0, :],
                    start=True, stop=False,
                )
                nc.tensor.matmul(
                    op[:, b, 65:130], pt[:, 256:384], vs[:, b * 2 + 1, :],
                    start=False, stop=True,
                )

            # single reciprocal + single broadcast-normalize for the macro
            op_v = op.rearrange("p b (n q) -> p (b n) q", q=65)  # [128,4,65]
            rec = rec_pool.tile([128, 4], F32, tag="rec")
            nc.vector.reciprocal(rec[:, :], op_v[:, :, 64])
            rec_b, opd_b = bass.broadcast_tensor_aps(
                rec.rearrange("p (n o) -> p n o", o=1), op_v[:, :, 0:64]
            )
            nc.vector.tensor_tensor(
                osb.rearrange("p b n h -> p (b n) h"), opd_b, rec_b,
                mybir.AluOpType.mult,
            )

        # all stores on SP's HWDGE ring: nc.scalar shares the ACT engine's
        # queue and ACT is the busiest engine
        nc.sync.dma_start(out_rm[it, :, :, :, :], osb[:, :, :, :])
        del state[it]

    if reps > 1:
        rep_ctx = tc.For_i(0, reps, 1)
        rep_ctx.__enter__()

    PIPELINE = os.environ.get("KERNEL_FLAT", "") != "1"
    if PIPELINE:
        stage_load(0)
        stage_load(1)
        stage_transpose(0)
        for i in range(N_MACRO):
            if i + 2 < N_MACRO:
                stage_load(i + 2)
            if i + 1 < N_MACRO:
                stage_transpose(i + 1)
            stage_compute(i)
    else:
        for i in range(N_MACRO):
            stage_load(i)
            stage_transpose(i)
            stage_compute(i)

    if reps > 1:
        rep_ctx.__exit__(None, None, None)


_CACHED = {}


def _build(reps=1, variant="full"):
    key = (reps, variant)
    if key in _CACHED:
        return _CACHED[key]
    nc = bacc.Bacc(
        "TRN2",
        target_bir_lowering=False,
        debug=False,
        num_devices=N_CORES,
        dynamic_dma_scratch_size=int(
            os.environ.get("KERNEL_SCRATCH", "49152")
        ),
    )
    x_d = nc.dram_tensor("x", [B_LOC, T, C], F32, kind="ExternalInput").ap()
    wq_d = nc.dram_tensor("Wq", [C, H], F32, kind="ExternalInput").ap()
    wk_d = nc.dram_tensor("Wk", [C, H], F32, kind="ExternalInput").ap()
    wv_d = nc.dram_tensor("Wv", [C, H], F32, kind="ExternalInput").ap()
    out_d = nc.dram_tensor("out", [B_LOC, T, H], F32, kind="ExternalOutput").ap()
    with tile.TileContext(nc) as tc, ExitStack() as ctx:
        build_attention_kernel(
            ctx, tc, out_d, x_d, wq_d, wk_d, wv_d, reps=reps, variant=variant
        )
    nc.compile()
    _CACHED[key] = nc
    return nc


_RUNNER = {}


def _get_runner(reps=1, variant="full"):
    """Persistent jitted SPMD executor (compiles/loads the NEFF once)."""
    key = (reps, variant)
    if key in _RUNNER:
        return _RUNNER[key]

    import jax
    from jax.sharding import Mesh, PartitionSpec
    from jax.experimental.shard_map import shard_map
    from concourse import bass2jax

    nc = _build(reps, variant)
    bass2jax.install_neuronx_cc_hook()

    partition_name = (
        nc.partition_id_tensor.name if nc.partition_id_tensor else None
    )
    in_names, out_names, out_avals = [], [], []
    for alloc in nc.m.functions[0].allocations:
        if not isinstance(alloc, mybir.MemoryLocationSet):
            continue
        name = alloc.memorylocations[0].name
        if alloc.kind == "ExternalInput":
            if name != partition_name:
                in_names.append(name)
        elif alloc.kind == "ExternalOutput":
            out_names.append(name)
            out_avals.append(
                jax.core.ShapedArray(
                    tuple(alloc.tensor_shape), mybir.dt.np(alloc.dtype)
                )
            )
    n_params = len(in_names)
    all_in_names = in_names + out_names
    if partition_name is not None:
        all_in_names = all_in_names + [partition_name]

    def _body(*args):
        operands = list(args)
        if partition_name is not None:
            operands.append(bass2jax.partition_id_tensor())
        outs = bass2jax._bass_exec_p.bind(
            *operands,
            out_avals=tuple(out_avals),
            in_names=tuple(all_in_names),
            out_names=tuple(out_names),
            lowering_input_output_aliases=(),
            sim_require_finite=True,
            sim_require_nnan=True,
            nc=nc,
        )
        return tuple(outs)

    devices = jax.devices()[:N_CORES]
    mesh = Mesh(np.asarray(devices), ("core",))
    fn = jax.jit(
        shard_map(
            _body,
            mesh=mesh,
            in_specs=(PartitionSpec("core"),) * (n_params + len(out_names)),
            out_specs=(PartitionSpec("core"),) * len(out_names),
            check_rep=False,
        ),
        keep_unused=True,
    )
    zero_outs = [
        np.zeros((N_CORES * a.shape[0], *a.shape[1:]), a.dtype) for a in out_avals
    ]
    _RUNNER[key] = (fn, in_names, out_names, out_avals, zero_outs)
    return _RUNNER[key]


def _global_inputs(x, Wk, Wq, Wv):
    """Concatenated per-core inputs keyed by BIR input name."""
    reps = {
        "x": np.ascontiguousarray(x, dtype=np.float32),
        "Wq": np.tile(np.asarray(Wq, np.float32), (N_CORES, 1)),
        "Wk": np.tile(np.asarray(Wk, np.float32), (N_CORES, 1)),
        "Wv": np.tile(np.asarray(Wv, np.float32), (N_CORES, 1)),
    }
    return reps


def kernel(x, Wk, Wq, Wv):
    x = np.asarray(x, dtype=np.float32)
    fn, in_names, out_names, out_avals, zero_outs = _get_runner()
    gi = _global_inputs(x, Wk, Wq, Wv)
    args = [gi[n] for n in in_names] + zero_outs
    outs = fn(*args)
    out = np.asarray(outs[out_names.index("out")])
    return out.astype(np.float32)


# revision 36
# speedup vs baseline: 1.1647x; 1.1108x over previous
"""Causal single-head attention forward on 8 Trainium2 NeuronCores.

Problem: x:(512,256,384) f32, Wq/Wk/Wv:(384,64) f32
  k = x@Wk; q = x@Wq; v = x@Wv
  wei = softmax(mask(q k^T / sqrt(384)))
  out = wei @ v                       -> (512, 256, 64) f32

Strategy: pure data parallel over batch (64 batches/core, no collectives).
Per core, batches are processed in macro-iterations of 2 batches, software
pipelined three deep: iteration i issues load(i+2) / transpose(i+1) /
compute(i) so every engine's FIFO has next-macro prep ahead of this
macro's tail ops.

Per macro:
  - load x naturally [T-part, C-free] (one contiguous cast-DMA f32->bf16)
  - transpose on PE via REAL matmuls (x block stationary, identity
    streaming) -- regular matmuls engage FWL and keep the HAM clock warm,
    unlike transpose-mode (~275ns/blk and no warmth credit)
  - packed [Wq|Wk... actually Wq solo and [Wk|Wv]] projections as in:
    qT [64,512] and kvT [128,512] over 3 C-chunks
  - S computed TRANSPOSED: sT[k, q] = kT.T @ qT so exp(sT) = pT is
    directly the lhsT of the PV matmul (no P transpose needed)
  - causal: block (k1, q0) never computed; diagonal blocks masked by a
    0/1 upper-triangular multiply after exp (one DVE op per batch via a
    strided 2-block view)
  - softmax denominators via a ones-column appended to v (col 64 of the
    PV matmul accumulates row sums); both batches' PV land in ONE psum
    tile so a single reciprocal + single broadcast-multiply normalize
    the whole macro.
All matmuls bf16 inputs with fp32 PSUM accumulation.
"""

import os
from contextlib import ExitStack

import numpy as np

import concourse.bass as bass
import concourse.mybir as mybir
import concourse.tile as tile
from concourse import bacc
from concourse.bass_utils import run_bass_kernel_spmd
from concourse.masks import make_identity

B, T, C, H = 512, 256, 384, 64
N_CORES = 8
B_LOC = B // N_CORES          # 64 batches per core
MACRO = 2                     # batches per macro-iteration
N_MACRO = B_LOC // MACRO      # 32
NC_CHUNKS = C // 128          # 3
SCALE = float(C) ** -0.5

F32 = mybir.dt.float32
F32R = mybir.dt.float32r
BF16 = mybir.dt.bfloat16


def build_attention_kernel(ctx, tc, out_d, x_d, wq_d, wk_d, wv_d, reps=1,
                           variant="full"):
    nc = tc.nc
    pool = lambda *a, **k: ctx.enter_context(tc.tile_pool(*a, **k))

    # ---------------- constants / weights (once) ----------------
    cpool = pool(name="const", bufs=1)
    ident = cpool.tile([128, 128], BF16, tag="ident")
    make_identity(nc, ident[:, :])

    # upper-triangular (incl diag) 0/1 mask in bf16: mask[j, i] = 1 if i >= j
    maskT = cpool.tile([128, 128], BF16, tag="maskT")
    nc.gpsimd.memset(maskT[:, :], 1.0)
    nc.gpsimd.affine_select(
        out=maskT[:, :],
        in_=maskT[:, :],
        compare_op=mybir.AluOpType.is_ge,
        fill=0.0,
        base=0,
        pattern=[[1, 128]],       # iota = -j + i  (channel j, free i)
        channel_multiplier=-1,
    )

    # weights: DRAM (384,64) -> staging f32 [128, 3, 64] -> packed bf16
    wst = cpool.tile([128, 3 * NC_CHUNKS, 64], F32, tag="wstage")
    for i, wd in enumerate((wq_d, wk_d, wv_d)):
        nc.sync.dma_start(
            wst[:, i * NC_CHUNKS:(i + 1) * NC_CHUNKS, :],
            wd.rearrange("(n p) h -> p n h", p=128),
        )
    # bf16 weights [128(C within chunk), chunk, *]; Wq pre-scaled by C^-0.5.
    # wkv packs [Wk | Wv] along the output dim -> kv projection in one chain.
    wq = cpool.tile([128, NC_CHUNKS, 64], BF16, tag="wq")
    nc.vector.tensor_scalar_mul(wq[:, :, :], wst[:, 0:NC_CHUNKS, :], SCALE)
    wkv = cpool.tile([128, NC_CHUNKS, 128], BF16, tag="wkv")
    nc.vector.tensor_copy(wkv[:, :, 0:64], wst[:, NC_CHUNKS:2 * NC_CHUNKS, :])
    nc.vector.tensor_copy(wkv[:, :, 64:128], wst[:, 2 * NC_CHUNKS:3 * NC_CHUNKS, :])

    # ---------------- pools ----------------
    xf_pool = pool(name="xf", bufs=4)       # bf16 natural x
    xts_pool = pool(name="xts", bufs=9)     # bf16 xT in SBUF (3/macro)
    qs_pool = pool(name="qs", bufs=3)       # bf16 qT in SBUF [64, 512]
    kvs_pool = pool(name="kvs", bufs=3)     # bf16 [kT|vT] in SBUF [128, 512]
    vs_pool = pool(name="vs", bufs=3)       # bf16 v (+ones col)
    pt_pool = pool(name="pt", bufs=4)       # bf16 pT
    os_pool = pool(name="os", bufs=3)       # f32 out staging
    rec_pool = pool(name="rec", bufs=4)     # f32 reciprocal

    RD1_OPS = os.environ.get("KERNEL_RD1_OPS", "") == "1"

    MERGED_STP = os.environ.get("KERNEL_MERGED_STP", "1") == "1"

    xtp_pool = pool(name="xtp", bufs=2, space="PSUM")   # xT psum
    qp_pool = pool(name="qp", bufs=1, space="PSUM")     # qT psum [64, 512]
    kvp_pool = pool(name="kvp", bufs=1, space="PSUM")   # kvT psum [128, 512]
    # merged: both batches' sT in one 2-bank tile -> one exp + one mask op
    stp_pool = pool(name="stp", bufs=1 if MERGED_STP else 2, space="PSUM")
    if RD1_OPS:
        vop_pool = pool(name="vtp", bufs=1, space="PSUM")
        op_pool = pool(name="op", bufs=1, space="PSUM")
    else:
        # vtp and op share one double-buffered ring (lifetimes interleave)
        vop_pool = pool(name="vop", bufs=2, space="PSUM")
        op_pool = vop_pool

    x_rm = x_d.rearrange("(m b) (n p) c -> m p b n c", p=128, b=MACRO)
    out_rm = out_d.rearrange("(m b) (n p) h -> m p b n h", p=128, b=MACRO)

    SPLIT_DMA = os.environ.get("KERNEL_SPLIT_DMA", "") == "1"
    state = {}

    def stage_load(j):
        xf = xf_pool.tile([128, MACRO, 2, C], BF16, tag="xf")
        if SPLIT_DMA:
            for b in range(MACRO):
                nc.gpsimd.dma_start(xf[:, b, :, :], x_rm[j, :, b, :, :])
        else:
            nc.gpsimd.dma_start(xf[:, :, :, :], x_rm[j, :, :, :, :])
        state[j] = {"xf": xf}

    def stage_transpose(j):
        # x natural -> xT on PE as true matmuls (x stationary, I streaming)
        xf = state[j]["xf"]
        xts = []
        for c in range(NC_CHUNKS):
            xtp = xtp_pool.tile([128, 512], F32, tag="xtp")
            for b in range(MACRO):
                for t in range(2):
                    nc.tensor.matmul(
                        xtp[:, (b * 2 + t) * 128:(b * 2 + t) * 128 + 128],
                        xf[:, b, t, c * 128:(c + 1) * 128],
                        ident[:, :],
                        start=True, stop=True,
                    )
            xt = xts_pool.tile([128, 512], BF16, tag="xts")
            nc.scalar.copy(xt[:, :], xtp[:, :])
            xts.append(xt)
        state[j]["xts"] = xts

    def stage_compute(it):
        xts = state[it]["xts"]

        # ---- projections: qT [64, 512]; [kT|vT] packed [128, 512] ----
        qp = qp_pool.tile([64, 512], F32, tag="qp")
        kvp = kvp_pool.tile([128, 512], F32, tag="kvp")
        for c in range(NC_CHUNKS):
            nc.tensor.matmul(
                qp[:, :], wq[:, c, :], xts[c][:, :],
                start=(c == 0), stop=(c == NC_CHUNKS - 1),
            )
        for c in range(NC_CHUNKS):
            nc.tensor.matmul(
                kvp[:, :], wkv[:, c, :], xts[c][:, :],
                start=(c == 0), stop=(c == NC_CHUNKS - 1),
            )
        qs = qs_pool.tile([64, 512], BF16, tag="qs")
        nc.vector.tensor_copy(qs[:, :], qp[:, :])
        kvs = kvs_pool.tile([128, 512], BF16, tag="kvs")
        nc.vector.tensor_copy(kvs[:, :], kvp[:, :])

        # ---- v natural [128(T), 64] via matmul-transpose of the vT half ----
        vtp = vop_pool.tile([128, 256], F32, tag="vop")
        for q in range(4):  # q = b*2 + t
            nc.tensor.matmul(
                vtp[:, q * 64:(q + 1) * 64],
                kvs[64:128, q * 128:(q + 1) * 128],
                ident[64:128, 64:128],
                start=True, stop=True,
            )
        vs = vs_pool.tile([128, 4, 65], BF16, tag="vs")
        nc.vector.tensor_copy(
            vs[:, :, 0:64], vtp.rearrange("p (q h) -> p q h", h=64)
        )
        nc.vector.memset(vs[:, :, 64], 1.0)

        # ---- sT for both batches, then exp/mask/PV ----
        # merged mode: one [128, 2, 512] tile (2 psum banks, batch b in bank
        # b; cols 384:512 unused pad) so ONE exp and ONE mask op cover the
        # macro. sT: [0:256] = sT(k0, q0|q1); [256:384] = sT(k1, q1).
        stps = []
        if MERGED_STP and not RD1_OPS:
            stp_m = stp_pool.tile([128, 2, 512], F32, tag="stp")
            for b in range(MACRO):
                qcol = b * 256
                nc.tensor.matmul(
                    stp_m[:, b, 0:256],
                    kvs[0:64, qcol:qcol + 128],
                    qs[:, qcol:qcol + 256],
                    start=True, stop=True,
                )
                nc.tensor.matmul(
                    stp_m[:, b, 256:384],
                    kvs[0:64, qcol + 128:qcol + 256],
                    qs[:, qcol + 128:qcol + 256],
                    start=True, stop=True,
                )
        else:
            for b in range(MACRO):
                qcol = b * 256
                stp = stp_pool.tile([128, 384], F32, tag="stp")
                nc.tensor.matmul(
                    stp[:, 0:256],
                    kvs[0:64, qcol:qcol + 128],        # kT chunk k0 [64, 128]
                    qs[:, qcol:qcol + 256],            # qT (2 chunks) [64, 256]
                    start=True, stop=True,
                )
                nc.tensor.matmul(
                    stp[:, 256:384],
                    kvs[0:64, qcol + 128:qcol + 256],  # kT chunk k1
                    qs[:, qcol + 128:qcol + 256],      # qT chunk q1
                    start=True, stop=True,
                )
                stps.append(stp)

        osb = os_pool.tile([128, MACRO, 2, 64], F32, tag="os")
        if RD1_OPS:
            for b in range(MACRO):
                stp = stps[b]
                pt = pt_pool.tile([128, 384], BF16, tag="pt")
                nc.scalar.activation(
                    pt[:, :], stp[:, :], mybir.ActivationFunctionType.Exp
                )
                nc.gpsimd.tensor_mul(pt[:, 0:128], pt[:, 0:128], maskT[:, :])
                nc.gpsimd.tensor_mul(
                    pt[:, 256:384], pt[:, 256:384], maskT[:, :]
                )
                op = op_pool.tile([128, 130], F32, tag="op")
                nc.tensor.matmul(
                    op[:, 0:65], pt[:, 0:128], vs[:, b * 2 + 0, :],
                    start=True, stop=True,
                )
                nc.tensor.matmul(
                    op[:, 65:130], pt[:, 128:256], vs[:, b * 2 + 0, :],
                    start=True, stop=False,
                )
                nc.tensor.matmul(
                    op[:, 65:130], pt[:, 256:384], vs[:, b * 2 + 1, :],
                    start=False, stop=True,
                )
                rec = rec_pool.tile([128, 2], F32, tag="rec")
                nc.vector.reciprocal(
                    rec[:, :],
                    op.rearrange("p (n q) -> p n q", q=65)[:, :, 64],
                )
                nc.vector.tensor_scalar_mul(
                    osb[:, b, 0, :], op[:, 0:64], rec[:, 0:1]
                )
                nc.vector.tensor_scalar_mul(
                    osb[:, b, 1, :], op[:, 65:129], rec[:, 1:2]
                )
        elif MERGED_STP:
            # ONE exp + ONE mask op for the whole macro
            op = op_pool.tile([128, 2, 130], F32, tag="vop")
            ptm = pt_pool.tile([128, 2, 384], BF16, tag="pt")
            nc.scalar.activation(
                ptm[:, :, :], stp_m[:, :, 0:384],
                mybir.ActivationFunctionType.Exp,
            )
            ptv = ptm.rearrange("p b (n q) -> p b n q", q=128)[:, :, 0:3:2, :]
            mask_b, ptv_b = bass.broadcast_tensor_aps(
                maskT.rearrange("p (a c q) -> p a c q", a=1, c=1), ptv
            )
            nc.vector.tensor_tensor(ptv, ptv_b, mask_b, mybir.AluOpType.mult)
            for b in range(MACRO):
                nc.tensor.matmul(
                    op[:, b, 0:65], ptm[:, b, 0:128], vs[:, b * 2 + 0, :],
                    start=True, stop=True,
                )
                nc.tensor.matmul(
                    op[:, b, 65:130], ptm[:, b, 128:256], vs[:, b * 2 + 0, :],
                    start=True, stop=False,
                )
                nc.tensor.matmul(
                    op[:, b, 65:130], ptm[:, b, 256:384], vs[:, b * 2 + 1, :],
                    start=False, stop=True,
                )

            # single reciprocal + single broadcast-normalize for the macro
            op_v = op.rearrange("p b (n q) -> p (b n) q", q=65)  # [128,4,65]
            rec = rec_pool.tile([128, 4], F32, tag="rec")
            nc.vector.reciprocal(rec[:, :], op_v[:, :, 64])
            rec_b, opd_b = bass.broadcast_tensor_aps(
                rec.rearrange("p (n o) -> p n o", o=1), op_v[:, :, 0:64]
            )
            nc.vector.tensor_tensor(
                osb.rearrange("p b n h -> p (b n) h"), opd_b, rec_b,
                mybir.AluOpType.mult,
            )
        else:
            # both batches' PV outputs land in one psum tile [128, 2, 130]
            op = op_pool.tile([128, 2, 130], F32, tag="vop")
            for b in range(MACRO):
                stp = stps[b]
                # pT = exp(sT)  (no max subtraction: logits are O(+-3))
                pt = pt_pool.tile([128, 384], BF16, tag="pt")
                nc.scalar.activation(
                    pt[:, :], stp[:, :], mybir.ActivationFunctionType.Exp
                )
                # causal 0/1 masking of the two diagonal blocks. Pool is the
                # least-busy engine and the skewed loop issues its DMA-gen
                # work early, so masks don't queue behind it.
                if os.environ.get("KERNEL_DVE_MASK", "1") == "1":
                    ptv = pt.rearrange("p (n q) -> p n q", q=128)[:, 0:3:2, :]
                    mask_b, ptv_b = bass.broadcast_tensor_aps(
                        maskT.rearrange("p (o q) -> p o q", o=1), ptv
                    )
                    nc.vector.tensor_tensor(
                        ptv, ptv_b, mask_b, mybir.AluOpType.mult
                    )
                else:
                    nc.gpsimd.tensor_mul(
                        pt[:, 0:128], pt[:, 0:128], maskT[:, :]
                    )
                    nc.gpsimd.tensor_mul(
                        pt[:, 256:384], pt[:, 256:384], maskT[:, :]
                    )

                # out = pT.T @ [v|1]: cols 0:65 = q0 (k0 only), 65:130 = q1
                nc.tensor.matmul(
                    op[:, b, 0:65], pt[:, 0:128], vs[:, b * 2 + 0, :],
                    start=True, stop=True,
                )
                nc.tensor.matmul(
                    op[:, b, 65:130], pt[:, 128:256], vs[:, b * 2 + 0, :],
                    start=True, stop=False,
                )
                nc.tensor.matmul(
                    op[:, b, 65:130], pt[:, 256:384], vs[:, b * 2 + 1, :],
                    start=False, stop=True,
                )

            # single reciprocal + single broadcast-normalize for the macro
            op_v = op.rearrange("p b (n q) -> p (b n) q", q=65)  # [128,4,65]
            rec = rec_pool.tile([128, 4], F32, tag="rec")
            nc.vector.reciprocal(rec[:, :], op_v[:, :, 64])
            rec_b, opd_b = bass.broadcast_tensor_aps(
                rec.rearrange("p (n o) -> p n o", o=1), op_v[:, :, 0:64]
            )
            nc.vector.tensor_tensor(
                osb.rearrange("p b n h -> p (b n) h"), opd_b, rec_b,
                mybir.AluOpType.mult,
            )

        # all stores on SP's HWDGE ring: nc.scalar shares the ACT engine's
        # queue and ACT is the busiest engine
        nc.sync.dma_start(out_rm[it, :, :, :, :], osb[:, :, :, :])
        del state[it]

    if reps > 1:
        rep_ctx = tc.For_i(0, reps, 1)
        rep_ctx.__enter__()

    PIPELINE = os.environ.get("KERNEL_FLAT", "") != "1"
    if PIPELINE:
        stage_load(0)
        stage_load(1)
        stage_transpose(0)
        for i in range(N_MACRO):
            if i + 2 < N_MACRO:
                stage_load(i + 2)
            if i + 1 < N_MACRO:
                stage_transpose(i + 1)
            stage_compute(i)
    else:
        for i in range(N_MACRO):
            stage_load(i)
            stage_transpose(i)
            stage_compute(i)

    if reps > 1:
        rep_ctx.__exit__(None, None, None)


_CACHED = {}


def _build(reps=1, variant="full"):
    key = (reps, variant)
    if key in _CACHED:
        return _CACHED[key]
    nc = bacc.Bacc(
        "TRN2",
        target_bir_lowering=False,
        debug=False,
        num_devices=N_CORES,
        dynamic_dma_scratch_size=int(
            os.environ.get("KERNEL_SCRATCH", "49152")
        ),
    )
    x_d = nc.dram_tensor("x", [B_LOC, T, C], F32, kind="ExternalInput").ap()
    wq_d = nc.dram_tensor("Wq", [C, H], F32, kind="ExternalInput").ap()
    wk_d = nc.dram_tensor("Wk", [C, H], F32, kind="ExternalInput").ap()
    wv_d = nc.dram_tensor("Wv", [C, H], F32, kind="ExternalInput").ap()
    out_d = nc.dram_tensor("out", [B_LOC, T, H], F32, kind="ExternalOutput").ap()
    with tile.TileContext(nc) as tc, ExitStack() as ctx:
        build_attention_kernel(
            ctx, tc, out_d, x_d, wq_d, wk_d, wv_d, reps=reps, variant=variant
        )
    nc.compile()
    _CACHED[key] = nc
    return nc


_RUNNER = {}


def _get_runner(reps=1, variant="full"):
    """Persistent jitted SPMD executor (compiles/loads the NEFF once)."""
    key = (reps, variant)
    if key in _RUNNER:
        return _RUNNER[key]

    import jax
    from jax.sharding import Mesh, PartitionSpec
    from jax.experimental.shard_map import shard_map
    from concourse import bass2jax

    nc = _build(reps, variant)
    bass2jax.install_neuronx_cc_hook()

    partition_name = (
        nc.partition_id_tensor.name if nc.partition_id_tensor else None
    )
    in_names, out_names, out_avals = [], [], []
    for alloc in nc.m.functions[0].allocations:
        if not isinstance(alloc, mybir.MemoryLocationSet):
            continue
        name = alloc.memorylocations[0].name
        if alloc.kind == "ExternalInput":
            if name != partition_name:
                in_names.append(name)
        elif alloc.kind == "ExternalOutput":
            out_names.append(name)
            out_avals.append(
                jax.core.ShapedArray(
                    tuple(alloc.tensor_shape), mybir.dt.np(alloc.dtype)
                )
            )
    n_params = len(in_names)
    all_in_names = in_names + out_names
    if partition_name is not None:
        all_in_names = all_in_names + [partition_name]

    def _body(*args):
        operands = list(args)
        if partition_name is not None:
            operands.append(bass2jax.partition_id_tensor())
        outs = bass2jax._bass_exec_p.bind(
            *operands,
            out_avals=tuple(out_avals),
            in_names=tuple(all_in_names),
            out_names=tuple(out_names),
            lowering_input_output_aliases=(),
            sim_require_finite=True,
            sim_require_nnan=True,
            nc=nc,
        )
        return tuple(outs)

    devices = jax.devices()[:N_CORES]
    mesh = Mesh(np.asarray(devices), ("core",))
    fn = jax.jit(
        shard_map(
            _body,
            mesh=mesh,
            in_specs=(PartitionSpec("core"),) * (n_params + len(out_names)),
            out_specs=(PartitionSpec("core"),) * len(out_names),
            check_rep=False,
        ),
        keep_unused=True,
    )
    zero_outs = [
        np.zeros((N_CORES * a.shape[0], *a.shape[1:]), a.dtype) for a in out_avals
    ]
    _RUNNER[key] = (fn, in_names, out_names, out_avals, zero_outs)
    return _RUNNER[key]


def _global_inputs(x, Wk, Wq, Wv):
    """Concatenated per-core inputs keyed by BIR input name."""
    reps = {
        "x": np.ascontiguousarray(x, dtype=np.float32),
        "Wq": np.tile(np.asarray(Wq, np.float32), (N_CORES, 1)),
        "Wk": np.tile(np.asarray(Wk, np.float32), (N_CORES, 1)),
        "Wv": np.tile(np.asarray(Wv, np.float32), (N_CORES, 1)),
    }
    return reps


def kernel(x, Wk, Wq, Wv):
    x = np.asarray(x, dtype=np.float32)
    fn, in_names, out_names, out_avals, zero_outs = _get_runner()
    gi = _global_inputs(x, Wk, Wq, Wv)
    args = [gi[n] for n in in_names] + zero_outs
    outs = fn(*args)
    out = np.asarray(outs[out_names.index("out")])
    return out.astype(np.float32)


# revision 37
# speedup vs baseline: 1.2635x; 1.0849x over previous
"""Causal single-head attention forward on 8 Trainium2 NeuronCores.

Problem: x:(512,256,384) f32, Wq/Wk/Wv:(384,64) f32
  k = x@Wk; q = x@Wq; v = x@Wv
  wei = softmax(mask(q k^T / sqrt(384)))
  out = wei @ v                       -> (512, 256, 64) f32

Strategy: pure data parallel over batch (64 batches/core, no collectives).
Per core, batches are processed in macro-iterations of 2 batches, software
pipelined three deep: iteration i issues load(i+2) / transpose(i+1) /
compute(i) so every engine's FIFO has next-macro prep ahead of this
macro's tail ops.

Per macro:
  - load x naturally [T-part, C-free] (one contiguous cast-DMA f32->bf16)
  - transpose on PE via REAL matmuls (x block stationary, identity
    streaming) -- regular matmuls engage FWL and keep the HAM clock warm,
    unlike transpose-mode (~275ns/blk and no warmth credit)
  - packed [Wq|Wk... actually Wq solo and [Wk|Wv]] projections as in:
    qT [64,512] and kvT [128,512] over 3 C-chunks
  - S computed TRANSPOSED: sT[k, q] = kT.T @ qT so exp(sT) = pT is
    directly the lhsT of the PV matmul (no P transpose needed)
  - causal: block (k1, q0) never computed; diagonal blocks masked by a
    0/1 upper-triangular multiply after exp (one DVE op per batch via a
    strided 2-block view)
  - softmax denominators via a ones-column appended to v (col 64 of the
    PV matmul accumulates row sums); both batches' PV land in ONE psum
    tile so a single reciprocal + single broadcast-multiply normalize
    the whole macro.
All matmuls bf16 inputs with fp32 PSUM accumulation.
"""

import os
from contextlib import ExitStack

import numpy as np

import concourse.bass as bass
import concourse.mybir as mybir
import concourse.tile as tile
from concourse import bacc
from concourse.bass_utils import run_bass_kernel_spmd
from concourse.masks import make_identity

B, T, C, H = 512, 256, 384, 64
N_CORES = 8
B_LOC = B // N_CORES          # 64 batches per core
MACRO = 2                     # batches per macro-iteration
N_MACRO = B_LOC // MACRO      # 32
NC_CHUNKS = C // 128          # 3
SCALE = float(C) ** -0.5

F32 = mybir.dt.float32
F32R = mybir.dt.float32r
BF16 = mybir.dt.bfloat16


def build_attention_kernel(ctx, tc, out_d, x_d, wq_d, wk_d, wv_d, reps=1,
                           variant="full"):
    nc = tc.nc
    pool = lambda *a, **k: ctx.enter_context(tc.tile_pool(*a, **k))

    # ---------------- constants / weights (once) ----------------
    cpool = pool(name="const", bufs=1)
    ident = cpool.tile([128, 128], BF16, tag="ident")
    make_identity(nc, ident[:, :])

    # upper-triangular (incl diag) 0/1 mask in bf16: mask[j, i] = 1 if i >= j
    maskT = cpool.tile([128, 128], BF16, tag="maskT")
    nc.gpsimd.memset(maskT[:, :], 1.0)
    nc.gpsimd.affine_select(
        out=maskT[:, :],
        in_=maskT[:, :],
        compare_op=mybir.AluOpType.is_ge,
        fill=0.0,
        base=0,
        pattern=[[1, 128]],       # iota = -j + i  (channel j, free i)
        channel_multiplier=-1,
    )

    # weights: DRAM (384,64) -> staging f32 [128, 3, 64] -> packed bf16
    wst = cpool.tile([128, 3 * NC_CHUNKS, 64], F32, tag="wstage")
    for i, wd in enumerate((wq_d, wk_d, wv_d)):
        nc.sync.dma_start(
            wst[:, i * NC_CHUNKS:(i + 1) * NC_CHUNKS, :],
            wd.rearrange("(n p) h -> p n h", p=128),
        )
    # bf16 weights [128(C within chunk), chunk, *]; Wq pre-scaled by C^-0.5.
    # wkv packs [Wk | Wv] along the output dim -> kv projection in one chain.
    wq = cpool.tile([128, NC_CHUNKS, 64], BF16, tag="wq")
    nc.vector.tensor_scalar_mul(wq[:, :, :], wst[:, 0:NC_CHUNKS, :], SCALE)
    wkv = cpool.tile([128, NC_CHUNKS, 128], BF16, tag="wkv")
    nc.vector.tensor_copy(wkv[:, :, 0:64], wst[:, NC_CHUNKS:2 * NC_CHUNKS, :])
    nc.vector.tensor_copy(wkv[:, :, 64:128], wst[:, 2 * NC_CHUNKS:3 * NC_CHUNKS, :])

    # ---------------- pools ----------------
    xf_pool = pool(name="xf", bufs=4)       # bf16 natural x
    xts_pool = pool(name="xts", bufs=9)     # bf16 xT in SBUF (3/macro)
    qs_pool = pool(name="qs", bufs=3)       # bf16 qT in SBUF [64, 512]
    kvs_pool = pool(name="kvs", bufs=3)     # bf16 [kT|vT] in SBUF [128, 512]
    vs_pool = pool(name="vs", bufs=3)       # bf16 v (+ones col)
    pt_pool = pool(name="pt", bufs=4)       # bf16 pT
    os_pool = pool(name="os", bufs=3)       # f32 out staging
    rec_pool = pool(name="rec", bufs=4)     # f32 reciprocal

    RD1_OPS = os.environ.get("KERNEL_RD1_OPS", "") == "1"

    MERGED_STP = os.environ.get("KERNEL_MERGED_STP", "1") == "1"

    xtp_pool = pool(name="xtp", bufs=2, space="PSUM")   # xT psum
    qp_pool = pool(name="qp", bufs=1, space="PSUM")     # qT psum [64, 512]
    kvp_pool = pool(name="kvp", bufs=1, space="PSUM")   # kvT psum [128, 512]
    # merged: both batches' sT in one 2-bank tile -> one exp + one mask op
    stp_pool = pool(name="stp", bufs=1 if MERGED_STP else 2, space="PSUM")
    if RD1_OPS:
        vop_pool = pool(name="vtp", bufs=1, space="PSUM")
        op_pool = pool(name="op", bufs=1, space="PSUM")
    else:
        # vtp and op share one double-buffered ring (lifetimes interleave)
        vop_pool = pool(name="vop", bufs=2, space="PSUM")
        op_pool = vop_pool

    x_rm = x_d.rearrange("(m b) (n p) c -> m p b n c", p=128, b=MACRO)
    out_rm = out_d.rearrange("(m b) (n p) h -> m p b n h", p=128, b=MACRO)

    SPLIT_DMA = os.environ.get("KERNEL_SPLIT_DMA", "") == "1"
    state = {}

    def stage_load(j):
        xf = xf_pool.tile([128, MACRO, 2, C], BF16, tag="xf")
        if SPLIT_DMA:
            for b in range(MACRO):
                nc.gpsimd.dma_start(xf[:, b, :, :], x_rm[j, :, b, :, :])
        else:
            nc.gpsimd.dma_start(xf[:, :, :, :], x_rm[j, :, :, :, :])
        state[j] = {"xf": xf}

    def stage_transpose(j):
        # x natural -> xT on PE as true matmuls (x stationary, I streaming)
        xf = state[j]["xf"]
        xts = []
        for c in range(NC_CHUNKS):
            xtp = xtp_pool.tile([128, 512], F32, tag="xtp")
            for b in range(MACRO):
                for t in range(2):
                    nc.tensor.matmul(
                        xtp[:, (b * 2 + t) * 128:(b * 2 + t) * 128 + 128],
                        xf[:, b, t, c * 128:(c + 1) * 128],
                        ident[:, :],
                        start=True, stop=True,
                    )
            xt = xts_pool.tile([128, 512], BF16, tag="xts")
            nc.scalar.copy(xt[:, :], xtp[:, :])
            xts.append(xt)
        state[j]["xts"] = xts

    def stage_compute(it):
        xts = state[it]["xts"]

        # ---- projections: qT [64, 512]; [kT|vT] packed [128, 512] ----
        qp = qp_pool.tile([64, 512], F32, tag="qp")
        kvp = kvp_pool.tile([128, 512], F32, tag="kvp")
        # interleave the two accumulation chains: consecutive MMs alternate
        # psum banks and weight tiles, letting LDW prefetch overlap streams
        for c in range(NC_CHUNKS):
            nc.tensor.matmul(
                qp[:, :], wq[:, c, :], xts[c][:, :],
                start=(c == 0), stop=(c == NC_CHUNKS - 1),
            )
            nc.tensor.matmul(
                kvp[:, :], wkv[:, c, :], xts[c][:, :],
                start=(c == 0), stop=(c == NC_CHUNKS - 1),
            )
        qs = qs_pool.tile([64, 512], BF16, tag="qs")
        nc.vector.tensor_copy(qs[:, :], qp[:, :])
        kvs = kvs_pool.tile([128, 512], BF16, tag="kvs")
        nc.vector.tensor_copy(kvs[:, :], kvp[:, :])

        # ---- v natural [128(T), 64] via matmul-transpose of the vT half ----
        vtp = vop_pool.tile([128, 256], F32, tag="vop")
        for q in range(4):  # q = b*2 + t
            nc.tensor.matmul(
                vtp[:, q * 64:(q + 1) * 64],
                kvs[64:128, q * 128:(q + 1) * 128],
                ident[64:128, 64:128],
                start=True, stop=True,
            )
        vs = vs_pool.tile([128, 4, 65], BF16, tag="vs")
        nc.vector.tensor_copy(
            vs[:, :, 0:64], vtp.rearrange("p (q h) -> p q h", h=64)
        )
        nc.vector.memset(vs[:, :, 64], 1.0)

        # ---- sT for both batches, then exp/mask/PV ----
        # merged mode: one [128, 2, 512] tile (2 psum banks, batch b in bank
        # b; cols 384:512 unused pad) so ONE exp and ONE mask op cover the
        # macro. sT: [0:256] = sT(k0, q0|q1); [256:384] = sT(k1, q1).
        stps = []
        if MERGED_STP and not RD1_OPS:
            stp_m = stp_pool.tile([128, 2, 512], F32, tag="stp")
            for b in range(MACRO):
                qcol = b * 256
                nc.tensor.matmul(
                    stp_m[:, b, 0:256],
                    kvs[0:64, qcol:qcol + 128],
                    qs[:, qcol:qcol + 256],
                    start=True, stop=True,
                )
                nc.tensor.matmul(
                    stp_m[:, b, 256:384],
                    kvs[0:64, qcol + 128:qcol + 256],
                    qs[:, qcol + 128:qcol + 256],
                    start=True, stop=True,
                )
        else:
            for b in range(MACRO):
                qcol = b * 256
                stp = stp_pool.tile([128, 384], F32, tag="stp")
                nc.tensor.matmul(
                    stp[:, 0:256],
                    kvs[0:64, qcol:qcol + 128],        # kT chunk k0 [64, 128]
                    qs[:, qcol:qcol + 256],            # qT (2 chunks) [64, 256]
                    start=True, stop=True,
                )
                nc.tensor.matmul(
                    stp[:, 256:384],
                    kvs[0:64, qcol + 128:qcol + 256],  # kT chunk k1
                    qs[:, qcol + 128:qcol + 256],      # qT chunk q1
                    start=True, stop=True,
                )
                stps.append(stp)

        osb = os_pool.tile([128, MACRO, 2, 64], F32, tag="os")
        if RD1_OPS:
            for b in range(MACRO):
                stp = stps[b]
                pt = pt_pool.tile([128, 384], BF16, tag="pt")
                nc.scalar.activation(
                    pt[:, :], stp[:, :], mybir.ActivationFunctionType.Exp
                )
                nc.gpsimd.tensor_mul(pt[:, 0:128], pt[:, 0:128], maskT[:, :])
                nc.gpsimd.tensor_mul(
                    pt[:, 256:384], pt[:, 256:384], maskT[:, :]
                )
                op = op_pool.tile([128, 130], F32, tag="op")
                nc.tensor.matmul(
                    op[:, 0:65], pt[:, 0:128], vs[:, b * 2 + 0, :],
                    start=True, stop=True,
                )
                nc.tensor.matmul(
                    op[:, 65:130], pt[:, 128:256], vs[:, b * 2 + 0, :],
                    start=True, stop=False,
                )
                nc.tensor.matmul(
                    op[:, 65:130], pt[:, 256:384], vs[:, b * 2 + 1, :],
                    start=False, stop=True,
                )
                rec = rec_pool.tile([128, 2], F32, tag="rec")
                nc.vector.reciprocal(
                    rec[:, :],
                    op.rearrange("p (n q) -> p n q", q=65)[:, :, 64],
                )
                nc.vector.tensor_scalar_mul(
                    osb[:, b, 0, :], op[:, 0:64], rec[:, 0:1]
                )
                nc.vector.tensor_scalar_mul(
                    osb[:, b, 1, :], op[:, 65:129], rec[:, 1:2]
                )
        elif MERGED_STP:
            # ONE exp + ONE mask op for the whole macro
            op = op_pool.tile([128, 2, 130], F32, tag="vop")
            ptm = pt_pool.tile([128, 2, 384], BF16, tag="pt")
            nc.scalar.activation(
                ptm[:, :, :], stp_m[:, :, 0:384],
                mybir.ActivationFunctionType.Exp,
            )
            ptv = ptm.rearrange("p b (n q) -> p b n q", q=128)[:, :, 0:3:2, :]
            mask_b, ptv_b = bass.broadcast_tensor_aps(
                maskT.rearrange("p (a c q) -> p a c q", a=1, c=1), ptv
            )
            nc.vector.tensor_tensor(ptv, ptv_b, mask_b, mybir.AluOpType.mult)
            for b in range(MACRO):
                nc.tensor.matmul(
                    op[:, b, 0:65], ptm[:, b, 0:128], vs[:, b * 2 + 0, :],
                    start=True, stop=True,
                )
                nc.tensor.matmul(
                    op[:, b, 65:130], ptm[:, b, 128:256], vs[:, b * 2 + 0, :],
                    start=True, stop=False,
                )
                nc.tensor.matmul(
                    op[:, b, 65:130], ptm[:, b, 256:384], vs[:, b * 2 + 1, :],
                    start=False, stop=True,
                )

            # single reciprocal + single broadcast-normalize for the macro
            op_v = op.rearrange("p b (n q) -> p (b n) q", q=65)  # [128,4,65]
            rec = rec_pool.tile([128, 4], F32, tag="rec")
            nc.vector.reciprocal(rec[:, :], op_v[:, :, 64])
            rec_b, opd_b = bass.broadcast_tensor_aps(
                rec.rearrange("p (n o) -> p n o", o=1), op_v[:, :, 0:64]
            )
            nc.vector.tensor_tensor(
                osb.rearrange("p b n h -> p (b n) h"), opd_b, rec_b,
                mybir.AluOpType.mult,
            )
        else:
            # both batches' PV outputs land in one psum tile [128, 2, 130]
            op = op_pool.tile([128, 2, 130], F32, tag="vop")
            for b in range(MACRO):
                stp = stps[b]
                # pT = exp(sT)  (no max subtraction: logits are O(+-3))
                pt = pt_pool.tile([128, 384], BF16, tag="pt")
                nc.scalar.activation(
                    pt[:, :], stp[:, :], mybir.ActivationFunctionType.Exp
                )
                # causal 0/1 masking of the two diagonal blocks. Pool is the
                # least-busy engine and the skewed loop issues its DMA-gen
                # work early, so masks don't queue behind it.
                if os.environ.get("KERNEL_DVE_MASK", "1") == "1":
                    ptv = pt.rearrange("p (n q) -> p n q", q=128)[:, 0:3:2, :]
                    mask_b, ptv_b = bass.broadcast_tensor_aps(
                        maskT.rearrange("p (o q) -> p o q", o=1), ptv
                    )
                    nc.vector.tensor_tensor(
                        ptv, ptv_b, mask_b, mybir.AluOpType.mult
                    )
                else:
                    nc.gpsimd.tensor_mul(
                        pt[:, 0:128], pt[:, 0:128], maskT[:, :]
                    )
                    nc.gpsimd.tensor_mul(
                        pt[:, 256:384], pt[:, 256:384], maskT[:, :]
                    )

                # out = pT.T @ [v|1]: cols 0:65 = q0 (k0 only), 65:130 = q1
                nc.tensor.matmul(
                    op[:, b, 0:65], pt[:, 0:128], vs[:, b * 2 + 0, :],
                    start=True, stop=True,
                )
                nc.tensor.matmul(
                    op[:, b, 65:130], pt[:, 128:256], vs[:, b * 2 + 0, :],
                    start=True, stop=False,
                )
                nc.tensor.matmul(
                    op[:, b, 65:130], pt[:, 256:384], vs[:, b * 2 + 1, :],
                    start=False, stop=True,
                )

            # single reciprocal + single broadcast-normalize for the macro
            op_v = op.rearrange("p b (n q) -> p (b n) q", q=65)  # [128,4,65]
            rec = rec_pool.tile([128, 4], F32, tag="rec")
            nc.vector.reciprocal(rec[:, :], op_v[:, :, 64])
            rec_b, opd_b = bass.broadcast_tensor_aps(
                rec.rearrange("p (n o) -> p n o", o=1), op_v[:, :, 0:64]
            )
            nc.vector.tensor_tensor(
                osb.rearrange("p b n h -> p (b n) h"), opd_b, rec_b,
                mybir.AluOpType.mult,
            )

        # all stores on SP's HWDGE ring: nc.scalar shares the ACT engine's
        # queue and ACT is the busiest engine
        nc.sync.dma_start(out_rm[it, :, :, :, :], osb[:, :, :, :])
        del state[it]

    if reps > 1:
        rep_ctx = tc.For_i(0, reps, 1)
        rep_ctx.__enter__()

    PIPELINE = os.environ.get("KERNEL_FLAT", "") != "1"
    if PIPELINE:
        stage_load(0)
        stage_load(1)
        stage_transpose(0)
        for i in range(N_MACRO):
            if i + 2 < N_MACRO:
                stage_load(i + 2)
            if i + 1 < N_MACRO:
                stage_transpose(i + 1)
            stage_compute(i)
    else:
        for i in range(N_MACRO):
            stage_load(i)
            stage_transpose(i)
            stage_compute(i)

    if reps > 1:
        rep_ctx.__exit__(None, None, None)


_CACHED = {}


def _build(reps=1, variant="full"):
    key = (reps, variant)
    if key in _CACHED:
        return _CACHED[key]
    nc = bacc.Bacc(
        "TRN2",
        target_bir_lowering=False,
        debug=False,
        num_devices=N_CORES,
        dynamic_dma_scratch_size=int(
            os.environ.get("KERNEL_SCRATCH", "49152")
        ),
    )
    x_d = nc.dram_tensor("x", [B_LOC, T, C], F32, kind="ExternalInput").ap()
    wq_d = nc.dram_tensor("Wq", [C, H], F32, kind="ExternalInput").ap()
    wk_d = nc.dram_tensor("Wk", [C, H], F32, kind="ExternalInput").ap()
    wv_d = nc.dram_tensor("Wv", [C, H], F32, kind="ExternalInput").ap()
    out_d = nc.dram_tensor("out", [B_LOC, T, H], F32, kind="ExternalOutput").ap()
    with tile.TileContext(nc) as tc, ExitStack() as ctx:
        build_attention_kernel(
            ctx, tc, out_d, x_d, wq_d, wk_d, wv_d, reps=reps, variant=variant
        )
    nc.compile()
    _CACHED[key] = nc
    return nc


_RUNNER = {}


def _get_runner(reps=1, variant="full"):
    """Persistent jitted SPMD executor (compiles/loads the NEFF once)."""
    key = (reps, variant)
    if key in _RUNNER:
        return _RUNNER[key]

    import jax
    from jax.sharding import Mesh, PartitionSpec
    from jax.experimental.shard_map import shard_map
    from concourse import bass2jax

    nc = _build(reps, variant)
    bass2jax.install_neuronx_cc_hook()

    partition_name = (
        nc.partition_id_tensor.name if nc.partition_id_tensor else None
    )
    in_names, out_names, out_avals = [], [], []
    for alloc in nc.m.functions[0].allocations:
        if not isinstance(alloc, mybir.MemoryLocationSet):
            continue
        name = alloc.memorylocations[0].name
        if alloc.kind == "ExternalInput":
            if name != partition_name:
                in_names.append(name)
        elif alloc.kind == "ExternalOutput":
            out_names.append(name)
            out_avals.append(
                jax.core.ShapedArray(
                    tuple(alloc.tensor_shape), mybir.dt.np(alloc.dtype)
                )
            )
    n_params = len(in_names)
    all_in_names = in_names + out_names
    if partition_name is not None:
        all_in_names = all_in_names + [partition_name]

    def _body(*args):
        operands = list(args)
        if partition_name is not None:
            operands.append(bass2jax.partition_id_tensor())
        outs = bass2jax._bass_exec_p.bind(
            *operands,
            out_avals=tuple(out_avals),
            in_names=tuple(all_in_names),
            out_names=tuple(out_names),
            lowering_input_output_aliases=(),
            sim_require_finite=True,
            sim_require_nnan=True,
            nc=nc,
        )
        return tuple(outs)

    devices = jax.devices()[:N_CORES]
    mesh = Mesh(np.asarray(devices), ("core",))
    fn = jax.jit(
        shard_map(
            _body,
            mesh=mesh,
            in_specs=(PartitionSpec("core"),) * (n_params + len(out_names)),
            out_specs=(PartitionSpec("core"),) * len(out_names),
            check_rep=False,
        ),
        keep_unused=True,
    )
    zero_outs = [
        np.zeros((N_CORES * a.shape[0], *a.shape[1:]), a.dtype) for a in out_avals
    ]
    _RUNNER[key] = (fn, in_names, out_names, out_avals, zero_outs)
    return _RUNNER[key]


def _global_inputs(x, Wk, Wq, Wv):
    """Concatenated per-core inputs keyed by BIR input name."""
    reps = {
        "x": np.ascontiguousarray(x, dtype=np.float32),
        "Wq": np.tile(np.asarray(Wq, np.float32), (N_CORES, 1)),
        "Wk": np.tile(np.asarray(Wk, np.float32), (N_CORES, 1)),
        "Wv": np.tile(np.asarray(Wv, np.float32), (N_CORES, 1)),
    }
    return reps


def kernel(x, Wk, Wq, Wv):
    x = np.asarray(x, dtype=np.float32)
    fn, in_names, out_names, out_avals, zero_outs = _get_runner()
    gi = _global_inputs(x, Wk, Wq, Wv)
    args = [gi[n] for n in in_names] + zero_outs
    outs = fn(*args)
    out = np.asarray(outs[out_names.index("out")])
    return out.astype(np.float32)
